# revision 10
# baseline (speedup 1.0000x reference)
"""Trainium2 Bass kernel for DnlsLoss (non-local search + refine loss).

Host side shards queries over 8 cores as (batch b in {0,1}) x (query-row band
g in {0..3}), expands patch matrices with replicate/reflect padding, and the
device computes, per query, ranking scores for all 243 candidates via TensorE
matmuls (candidate norm folded in as an augmented patch row), selects the 9
best non-self candidates with VectorE max/match_replace, and accumulates the
refine distances of the selected candidates with a masked multiply-reduce.
"""

import sys

sys.path.insert(0, "/opt/trn_rl_repo")

import numpy as np
import ml_dtypes

# ---- problem constants (hardcoded per spec) ----
B, T, C, H, W = 2, 5, 3, 96, 96
WS, WT, PS, PSD, K, S0 = 9, 1, 5, 7, 10, 4
NH = H // S0  # 24 query rows
NW = W // S0  # 24 queries per row
NCORES = 8
NBANDS = 4  # query-row bands per batch
ROWS_PER_BAND = NH // NBANDS  # 6 qh rows per core
NPAIRS = 27  # (2*WT+1) * WS = 3 * 9  (dt, dh) pairs
WTILD = 104  # padded candidate column range cw~ in [-4, 99]
CH_BAND = ROWS_PER_BAND * S0 - S0 + 1 + 8  # 29 candidate rows per core
D5 = C * PS * PS  # 75
D7 = C * PSD * PSD  # 147
WIDE = NPAIRS * WTILD  # 2808 -> but stored per 4-pair bank: 7 banks * 416
BANKS = 7  # ceil(27 / 4)
BANKW = 416  # 4 pairs * 104 cols
XW = NPAIRS * WTILD  # 2808 flat (s, w~) width of x
NTILES = 8  # 8 tiles of 4 query rows (rows 30,31 duplicate 28,29)
ROWS_PER_TILE = 4
NEG_BIG = -1.0e30
NEG_ZAP = -3.0e30
NEG_THR = -2.0e30


def _refl(i, n):
    i = np.where(i < 0, -i, i)
    return np.where(i >= n, 2 * (n - 1) - i, i)


def _patch_lut(n, ps):
    """centers 0..n-1 -> reflect-resolved source index [n, ps]."""
    off = np.arange(ps) - ps // 2
    return _refl(np.arange(n)[:, None] + off[None, :], n)


def _expand_patches(img, ps):
    """img [T,C,H,W] -> patches [C*ps*ps, T, H, W] (reflect bounds)."""
    hh = _patch_lut(H, ps)  # [H, ps]
    ww = _patch_lut(W, ps)  # [W, ps]
    p = img[:, :, hh, :]  # [T,C,H,ps,W]
    p = p[:, :, :, :, ww]  # [T,C,H,ps,W,ps]
    p = np.transpose(p, (1, 3, 5, 0, 2, 4))  # [C,ps,ps,T,H,W]
    return p.reshape(C * ps * ps, T, H, W)


def _core_rows(g):
    """30 (qt, qh) rows for band g, +2 duplicated -> 32 rows, 8 tiles of 4."""
    qhs = [24 * g + 4 * j for j in range(ROWS_PER_BAND)]
    rows = [(qt, qh) for qt in range(T) for qh in qhs]
    rows += rows[28:30]  # pad tile 8 with duplicates of rows 28,29
    return rows


def _host_prep():
    """Builds per-core input maps. Returns (in_maps, nq7_sums)."""
    # placeholder; filled by kernel()
    raise NotImplementedError


def build_core_inputs(noisy, deno, b, g):
    """All shard tensors for core (b, g)."""
    S = noisy[b]  # [T,C,H,W]
    Dn = deno[b]

    pat5 = _expand_patches(S, PS)  # [75, T, H, W]
    pat7n = _expand_patches(S, PSD)  # [147, T, H, W]
    pat7d = _expand_patches(Dn, PSD)  # [147, T, H, W]
    n5 = np.sum(pat5 * pat5, axis=0)  # [T, H, W]
    n7 = np.sum(pat7n * pat7n, axis=0)  # [T, H, W]

    # candidate band: ch~ in [24g-4, 24g+24] (29 rows), cw~ in [-4, 99] (104)
    ch_idx = np.clip(24 * g - 4 + np.arange(CH_BAND), 0, H - 1)  # [29]
    cw_idx = np.clip(np.arange(WTILD) - 4, 0, W - 1)  # [104]

    # p5n_aug [76, T, 29, 104] bf16: rows 0..74 patch, row 75 = N5
    p5n = pat5[:, :, ch_idx, :][:, :, :, cw_idx]  # [75, T, 29, 104]
    p5n_aug = np.concatenate(
        [p5n, n5[None, :, ch_idx, :][:, :, :, cw_idx]], axis=0
    )  # [76, T, 29, 104]

    # p7n aug [148, ...] split 128 + 20; row 147 = N7
    p7n = pat7n[:, :, ch_idx, :][:, :, :, cw_idx]  # [147, T, 29, 104]
    p7n_aug = np.concatenate(
        [p7n, n7[None, :, ch_idx, :][:, :, :, cw_idx]], axis=0
    )  # [148, T, 29, 104]

    # query-side lhsT tiles [*, NTILES, 128]
    rows = _core_rows(g)
    pq = np.zeros((D5 + 1, NTILES, 128), np.float32)
    pdA = np.zeros((128, NTILES, 128), np.float32)
    pdB = np.zeros((20, NTILES, 128), np.float32)
    nq7 = np.zeros((NTILES, 128), np.float32)
    for t in range(NTILES):
        for r in range(ROWS_PER_TILE):
            qt, qh = rows[t * ROWS_PER_TILE + r]
            cols = slice(32 * r, 32 * r + NW)
            q5 = pat5[:, qt, qh, 0 : W : S0]  # [75, 24]
            q7 = pat7d[:, qt, qh, 0 : W : S0]  # [147, 24]
            pq[:D5, t, cols] = 2.0 * q5
            pq[D5, t, cols] = -1.0
            pdA[:, t, cols] = -2.0 * q7[:128]
            pdB[:19, t, cols] = -2.0 * q7[128:]
            pdB[19, t, cols] = 1.0
            nq7[t, cols] = np.sum(q7 * q7, axis=0)

    # sum of 9 * ||deno7 query patch||^2 over the core's REAL queries
    validq = np.zeros((128, 2), np.float32)
    for p in range(128):
        validq[p, 0] = 1.0 if (p % 32) < NW else 0.0
        validq[p, 1] = 1.0 if (p < 64 and (p % 32) < NW) else 0.0
    vq_t = np.concatenate(
        [np.tile(validq[:, 0], (7, 1)), validq[None, :, 1]], axis=0
    )  # [NTILES, 128]
    nq7_sum = float(np.sum(nq7 * vq_t) * (K - 1))

    # maskbig [128, 2912(banked)] f32: flat (s, w~) layout col = 104*s + w~,
    # stored bank-contiguous (4 pairs / 416 cols per bank) == same flat order.
    mask = np.full((128, XW), NEG_BIG, np.float32)
    for p in range(128):
        i = p % 32
        if i >= NW:
            continue
        for s in range(NPAIRS):
            w0 = 4 * i  # w~ = 4i + dw + 4, dw in [-4, 4] -> w~ in [4i, 4i+8]
            mask[p, 104 * s + w0 : 104 * s + w0 + 9] = 0.0
        mask[p, 104 * 13 + 4 * i + 4] = NEG_BIG  # self slot (dt=0, dh=0, dw=0)

    return {
        "p5n": p5n_aug.astype(ml_dtypes.bfloat16),
        "p7na": np.ascontiguousarray(p7n_aug[:128]).astype(ml_dtypes.bfloat16),
        "p7nb": np.ascontiguousarray(p7n_aug[128:]).astype(ml_dtypes.bfloat16),
        "pq": pq.astype(ml_dtypes.bfloat16),
        "pda": pdA.astype(ml_dtypes.bfloat16),
        "pdb": pdB.astype(ml_dtypes.bfloat16),
        "maskbig": mask,
        "validq": validq,
    }, nq7_sum


# ------------------------------------------------------------------
# matmul run decomposition (python-static): for bank gbank, row (qt, qh),
# yield (dt_idx, s0, s1) contiguous pair-slot runs with the same dt.
def _bank_runs(gbank):
    s_lo, s_hi = 4 * gbank, min(4 * gbank + 4, NPAIRS)
    runs = []
    s = s_lo
    while s < s_hi:
        dt_idx = s // 9
        e = min(s_hi, 9 * (dt_idx + 1))
        runs.append((dt_idx, s, e))
        s = e
    return runs


def build_bass_program():
    import concourse.bass as bass
    import concourse.tile as tile
    from concourse import mybir

    fp32 = mybir.dt.float32
    bf16 = mybir.dt.bfloat16

    nc = bass.Bass()
    d_p5n = nc.declare_dram_parameter("p5n", [D5 + 1, T, CH_BAND, WTILD], bf16, isOutput=False)
    d_p7na = nc.declare_dram_parameter("p7na", [128, T, CH_BAND, WTILD], bf16, isOutput=False)
    d_p7nb = nc.declare_dram_parameter("p7nb", [20, T, CH_BAND, WTILD], bf16, isOutput=False)
    d_pq = nc.declare_dram_parameter("pq", [D5 + 1, NTILES, 128], bf16, isOutput=False)
    d_pda = nc.declare_dram_parameter("pda", [128, NTILES, 128], bf16, isOutput=False)
    d_pdb = nc.declare_dram_parameter("pdb", [20, NTILES, 128], bf16, isOutput=False)
    d_mask = nc.declare_dram_parameter("maskbig", [128, XW], fp32, isOutput=False)
    d_vq = nc.declare_dram_parameter("validq", [128, 2], fp32, isOutput=False)
    d_out = nc.declare_dram_parameter("out", [1, 1], fp32, isOutput=True)

    rows = _core_rows(0)  # qt/ch~-geometry identical across cores (band-local)

    with tile.TileContext(nc) as tc:
        with (
            tc.tile_pool(name="const", bufs=1) as cpool,
            tc.tile_pool(name="psum", bufs=8, space="PSUM") as pspool,
            tc.tile_pool(name="work", bufs=2) as wpool,
            tc.tile_pool(name="small", bufs=2) as spool,
        ):
            # ---- resident loads ----
            p5n = cpool.tile([D5 + 1, T, CH_BAND, WTILD], bf16)
            nc.sync.dma_start(out=p5n[:], in_=d_p5n[:])
            p7na = cpool.tile([128, T, CH_BAND, WTILD], bf16)
            nc.sync.dma_start(out=p7na[:], in_=d_p7na[:])
            # the K-split pair (p7nb weights + rhs) parked at partitions 96..115
            # (matmul requires Fmap and Weight to share the base partition)
            p7nb_h = cpool.tile([116, T, CH_BAND, WTILD], bf16)
            p7nb = p7nb_h[96:116]
            nc.sync.dma_start(out=p7nb[:], in_=d_p7nb[:])
            pq = cpool.tile([D5 + 1, NTILES, 128], bf16)
            nc.sync.dma_start(out=pq[:], in_=d_pq[:])
            pda = cpool.tile([128, NTILES, 128], bf16)
            nc.sync.dma_start(out=pda[:], in_=d_pda[:])
            pdb_h = cpool.tile([116, NTILES, 128], bf16)
            pdb = pdb_h[96:116]
            nc.sync.dma_start(out=pdb[:], in_=d_pdb[:])
            maskbig = cpool.tile([128, XW], fp32)
            nc.sync.dma_start(out=maskbig[:], in_=d_mask[:])

            # one consolidated scratch tile (pool pads each tile to 4KB)
            scr = cpool.tile([128, 32], fp32)
            maxv = scr[:, 0:8]
            m98 = scr[:, 8:16]   # col 8 = m9, cols 9..15 = NEG_ZAP filler
            accs = [scr[:, 16 + g : 17 + g] for g in range(BANKS)]
            total = scr[:, 23:24]
            ones = scr[:, 24:25]
            vq = scr[:, 25:27]
            nc.sync.dma_start(out=vq[:], in_=d_vq[:])
            nc.vector.memset(total[:], 0.0)
            nc.vector.memset(ones[:], 1.0)

            x = cpool.tile([128, XW], fp32)

            for ti in range(NTILES):
                # ---------- search scores into PSUM ----------
                ps_banks = []
                for gb in range(BANKS):
                    npair = min(4 * gb + 4, NPAIRS) - 4 * gb
                    psb = pspool.tile([128, BANKW], fp32, tag="ps")
                    ps_banks.append((psb, npair))
                    for r in range(ROWS_PER_TILE):
                        qt, qh = rows[ti * ROWS_PER_TILE + r]
                        j = (qh % 24) // 4  # band-local row index
                        for dt_idx, s0, s1 in _bank_runs(gb):
                            tprime = min(max(qt + dt_idx - 1, 0), T - 1)
                            lch0 = 4 * j + (s0 - 9 * dt_idx)
                            nrun = s1 - s0
                            rhs = p5n[:, tprime, lch0 : lch0 + nrun, :]
                            out = psb[32 * r : 32 * r + 32,
                                      (s0 - 4 * gb) * WTILD : (s1 - 4 * gb) * WTILD]
                            nc.tensor.matmul(
                                out, pq[:, ti, 32 * r : 32 * r + 32], rhs,
                                start=True, stop=True, tile_position=(0, 32 * r),
                            )
                    # ---------- evacuate + mask ----------
                    xs = x[:, 4 * gb * WTILD : (4 * gb + npair) * WTILD]
                    ms = maskbig[:, 4 * gb * WTILD : (4 * gb + npair) * WTILD]
                    nc.vector.tensor_tensor(
                        out=xs, in0=psb[:, 0 : npair * WTILD], in1=ms,
                        op=mybir.AluOpType.add,
                    )

                # ---------- top-9 selection on x ----------
                nc.vector.max(out=maxv[:], in_=x[:])
                nc.vector.match_replace(
                    out=x[:], in_to_replace=maxv[:], in_values=x[:],
                    imm_value=NEG_ZAP,
                )
                nc.vector.memset(m98[:], NEG_ZAP)
                nc.vector.tensor_reduce(
                    out=m98[:, 0:1], in_=x[:], axis=mybir.AxisListType.X,
                    op=mybir.AluOpType.max,
                )
                nc.vector.match_replace(
                    out=x[:], in_to_replace=m98[:], in_values=x[:],
                    imm_value=NEG_ZAP,
                )
                # in place: x becomes the 0/1 selection mask
                nc.vector.tensor_scalar(
                    x[:], x[:], NEG_THR, None, mybir.AluOpType.is_lt
                )

                # ---------- refine scores + masked accumulate ----------
                for gb in range(BANKS):
                    npair = min(4 * gb + 4, NPAIRS) - 4 * gb
                    prb = pspool.tile([128, BANKW], fp32, tag="ps")
                    for r in range(ROWS_PER_TILE):
                        qt, qh = rows[ti * ROWS_PER_TILE + r]
                        j = (qh % 24) // 4
                        for dt_idx, s0, s1 in _bank_runs(gb):
                            tprime = min(max(qt + dt_idx - 1, 0), T - 1)
                            lch0 = 4 * j + (s0 - 9 * dt_idx)
                            nrun = s1 - s0
                            rhsA = p7na[:, tprime, lch0 : lch0 + nrun, :]
                            rhsB = p7nb[:, tprime, lch0 : lch0 + nrun, :]
                            out = prb[32 * r : 32 * r + 32,
                                      (s0 - 4 * gb) * WTILD : (s1 - 4 * gb) * WTILD]
                            lhsA = pda[:, ti, 32 * r : 32 * r + 32]
                            lhsB = pdb[:, ti, 32 * r : 32 * r + 32]
                            nc.tensor.matmul(out, lhsA, rhsA, start=True,
                                             stop=False, tile_position=(0, 32 * r))
                            nc.tensor.matmul(out, lhsB, rhsB, start=False,
                                             stop=True, tile_position=(96, 32 * r))
                    sel = x[:, 4 * gb * WTILD : (4 * gb + npair) * WTILD]
                    nc.vector.scalar_tensor_tensor(
                        out=sel,
                        in0=sel,
                        scalar=1.0,
                        in1=prb[:, 0 : npair * WTILD],
                        op0=mybir.AluOpType.bypass,
                        op1=mybir.AluOpType.mult,
                        accum_out=accs[gb][:],
                    )

                # total += sum_g(acc_g) * validq(tile variant)
                accsum = scr[:, 27:28]
                nc.vector.tensor_reduce(
                    out=accsum, in_=scr[:, 16 : 16 + BANKS],
                    axis=mybir.AxisListType.X, op=mybir.AluOpType.add,
                )
                v = vq[:, 1:2] if ti == NTILES - 1 else vq[:, 0:1]
                nc.vector.scalar_tensor_tensor(
                    out=total[:], in0=accsum, scalar=v,
                    in1=total[:], op0=mybir.AluOpType.mult,
                    op1=mybir.AluOpType.add,
                )

            # ---- final partition reduce: out[0,0] = sum_p total[p] ----
            fin = pspool.tile([128, BANKW], fp32, tag="ps")
            nc.tensor.matmul(fin[0:1, 0:1], ones[:], total[:], start=True, stop=True)
            osb = spool.tile([1, 1], fp32, tag="osb")
            nc.vector.tensor_copy(out=osb[:], in_=fin[0:1, 0:1])
            nc.sync.dma_start(out=d_out[:], in_=osb[:])

    return nc


_CACHED = {}


def _split_multiwaits(bir_bytes):
    """This walrus build supports one sem-wait per instruction; Tile emits
    several. Split extras onto NoOps inserted immediately before (same
    engine, program order preserved -> semantically identical)."""
    import json

    m = json.loads(bir_bytes)
    ctr = [0]
    for fn in m["functions"]:
        for bb in fn["blocks"]:
            out = []
            for ins in bb["instructions"]:
                si = ins.get("sync_info")
                waits = (si or {}).get("on_wait") or []
                for w in waits[:-1]:
                    ctr[0] += 1
                    out.append(
                        {
                            "debug": ins.get("debug", 0),
                            "engine": ins["engine"],
                            "ins": [],
                            "outs": [],
                            "name": f"I-mwsplit{ctr[0]}",
                            "opcode": "NoOp",
                            "sync_info": {"on_update": [], "on_wait": [w]},
                        }
                    )
                if len(waits) > 1:
                    si["on_wait"] = [waits[-1]]
                out.append(ins)
            bb["instructions"] = out
    return json.dumps(m).encode()


def _install_multiwait_patch():
    if _CACHED.get("mw_patch"):
        return
    from concourse import bass2jax, bass_utils

    orig = bass_utils.compile_bir_kernel

    def wrapper(bir_json, tmpdir, neff_name="file.neff"):
        return orig(_split_multiwaits(bir_json), tmpdir, neff_name=neff_name)

    bass2jax.compile_bir_kernel = wrapper
    _CACHED["mw_patch"] = True


def kernel(noisy, deno, curr_epoch=None):
    noisy = np.asarray(noisy, np.float32)
    deno = np.asarray(deno, np.float32)

    in_maps = []
    nq7_sums = []
    for core in range(NCORES):
        b, g = core % B, core // B
        m, nq7s = build_core_inputs(noisy, deno, b, g)
        in_maps.append(m)
        nq7_sums.append(nq7s)

    if "nc" not in _CACHED:
        _CACHED["nc"] = build_bass_program()
    nc = _CACHED["nc"]

    _install_multiwait_patch()
    from concourse.bass_utils import run_bass_kernel_spmd

    res = run_bass_kernel_spmd(nc, in_maps, list(range(NCORES)))
    parts = [float(res.results[i]["out"].reshape(-1)[0]) for i in range(NCORES)]
    loss = (sum(parts) + sum(nq7_sums)) / (B * T * NH * NW * (K - 1))
    return np.float32(loss)


# revision 13
# speedup vs baseline: 409.1864x; 409.1864x over previous
"""Trainium2 Bass kernel for DnlsLoss (non-local search + refine loss).

Host side shards queries over 8 cores as (batch b in {0,1}) x (query-row band
g in {0..3}), expands patch matrices with replicate/reflect padding, and the
device computes, per query, ranking scores for all 243 candidates via TensorE
matmuls (candidate norm folded in as an augmented patch row), selects the 9
best non-self candidates with VectorE max/match_replace, and accumulates the
refine distances of the selected candidates with a masked multiply-reduce.
"""

import sys

sys.path.insert(0, "/opt/trn_rl_repo")

import numpy as np
import ml_dtypes

# ---- problem constants (hardcoded per spec) ----
B, T, C, H, W = 2, 5, 3, 96, 96
WS, WT, PS, PSD, K, S0 = 9, 1, 5, 7, 10, 4
NH = H // S0  # 24 query rows
NW = W // S0  # 24 queries per row
NCORES = 8
NBANDS = 4  # query-row bands per batch
ROWS_PER_BAND = NH // NBANDS  # 6 qh rows per core
NPAIRS = 27  # (2*WT+1) * WS = 3 * 9  (dt, dh) pairs
WTILD = 104  # padded candidate column range cw~ in [-4, 99]
CH_BAND = ROWS_PER_BAND * S0 - S0 + 1 + 8  # 29 candidate rows per core
D5 = C * PS * PS  # 75
D7 = C * PSD * PSD  # 147
WIDE = NPAIRS * WTILD  # 2808 -> but stored per 4-pair bank: 7 banks * 416
BANKS = 7  # ceil(27 / 4)
BANKW = 416  # 4 pairs * 104 cols
XW = NPAIRS * WTILD  # 2808 flat (s, w~) width of x
NTILES = 8  # 8 tiles of 4 query rows (rows 30,31 duplicate 28,29)
ROWS_PER_TILE = 4
NEG_BIG = -1.0e30
NEG_ZAP = -3.0e30
NEG_THR = -2.0e30


def _refl(i, n):
    i = np.where(i < 0, -i, i)
    return np.where(i >= n, 2 * (n - 1) - i, i)


def _patch_lut(n, ps):
    """centers 0..n-1 -> reflect-resolved source index [n, ps]."""
    off = np.arange(ps) - ps // 2
    return _refl(np.arange(n)[:, None] + off[None, :], n)


def _expand_patches(img, ps):
    """img [T,C,H,W] -> patches [C*ps*ps, T, H, W] (reflect bounds)."""
    hh = _patch_lut(H, ps)  # [H, ps]
    ww = _patch_lut(W, ps)  # [W, ps]
    p = img[:, :, hh, :]  # [T,C,H,ps,W]
    p = p[:, :, :, :, ww]  # [T,C,H,ps,W,ps]
    p = np.transpose(p, (1, 3, 5, 0, 2, 4))  # [C,ps,ps,T,H,W]
    return p.reshape(C * ps * ps, T, H, W)


def _core_rows(g):
    """30 (qt, qh) rows for band g, +2 duplicated -> 32 rows, 8 tiles of 4."""
    qhs = [24 * g + 4 * j for j in range(ROWS_PER_BAND)]
    rows = [(qt, qh) for qt in range(T) for qh in qhs]
    rows += rows[28:30]  # pad tile 8 with duplicates of rows 28,29
    return rows


def _host_prep():
    """Builds per-core input maps. Returns (in_maps, nq7_sums)."""
    # placeholder; filled by kernel()
    raise NotImplementedError


def build_core_inputs(noisy, deno, b, g):
    """All shard tensors for core (b, g)."""
    S = noisy[b]  # [T,C,H,W]
    Dn = deno[b]

    pat5 = _expand_patches(S, PS)  # [75, T, H, W]
    pat7n = _expand_patches(S, PSD)  # [147, T, H, W]
    pat7d = _expand_patches(Dn, PSD)  # [147, T, H, W]
    n5 = np.sum(pat5 * pat5, axis=0)  # [T, H, W]
    n7 = np.sum(pat7n * pat7n, axis=0)  # [T, H, W]

    # candidate band: ch~ in [24g-4, 24g+24] (29 rows), cw~ in [-4, 99] (104)
    ch_idx = np.clip(24 * g - 4 + np.arange(CH_BAND), 0, H - 1)  # [29]
    cw_idx = np.clip(np.arange(WTILD) - 4, 0, W - 1)  # [104]

    # p5n_aug [76, T, 29, 104] bf16: rows 0..74 patch, row 75 = N5
    p5n = pat5[:, :, ch_idx, :][:, :, :, cw_idx]  # [75, T, 29, 104]
    p5n_aug = np.concatenate(
        [p5n, n5[None, :, ch_idx, :][:, :, :, cw_idx]], axis=0
    )  # [76, T, 29, 104]

    # p7n aug [148, ...] split 128 + 20; row 147 = N7
    p7n = pat7n[:, :, ch_idx, :][:, :, :, cw_idx]  # [147, T, 29, 104]
    p7n_aug = np.concatenate(
        [p7n, n7[None, :, ch_idx, :][:, :, :, cw_idx]], axis=0
    )  # [148, T, 29, 104]

    # query-side lhsT tiles [*, NTILES, 128]
    rows = _core_rows(g)
    pq = np.zeros((D5 + 1, NTILES, 128), np.float32)
    pdA = np.zeros((128, NTILES, 128), np.float32)
    pdB = np.zeros((20, NTILES, 128), np.float32)
    nq7 = np.zeros((NTILES, 128), np.float32)
    for t in range(NTILES):
        for r in range(ROWS_PER_TILE):
            qt, qh = rows[t * ROWS_PER_TILE + r]
            cols = slice(32 * r, 32 * r + NW)
            q5 = pat5[:, qt, qh, 0 : W : S0]  # [75, 24]
            q7 = pat7d[:, qt, qh, 0 : W : S0]  # [147, 24]
            pq[:D5, t, cols] = 2.0 * q5
            pq[D5, t, cols] = -1.0
            pdA[:, t, cols] = -2.0 * q7[:128]
            pdB[:19, t, cols] = -2.0 * q7[128:]
            pdB[19, t, cols] = 1.0
            nq7[t, cols] = np.sum(q7 * q7, axis=0)

    # sum of 9 * ||deno7 query patch||^2 over the core's REAL queries
    validq = np.zeros((128, 2), np.float32)
    for p in range(128):
        validq[p, 0] = 1.0 if (p % 32) < NW else 0.0
        validq[p, 1] = 1.0 if (p < 64 and (p % 32) < NW) else 0.0
    vq_t = np.concatenate(
        [np.tile(validq[:, 0], (7, 1)), validq[None, :, 1]], axis=0
    )  # [NTILES, 128]
    nq7_sum = float(np.sum(nq7 * vq_t) * (K - 1))

    # maskbig [128, 2912(banked)] f32: flat (s, w~) layout col = 104*s + w~,
    # stored bank-contiguous (4 pairs / 416 cols per bank) == same flat order.
    mask = np.full((128, XW), NEG_BIG, np.float32)
    for p in range(128):
        i = p % 32
        if i >= NW:
            continue
        for s in range(NPAIRS):
            w0 = 4 * i  # w~ = 4i + dw + 4, dw in [-4, 4] -> w~ in [4i, 4i+8]
            mask[p, 104 * s + w0 : 104 * s + w0 + 9] = 0.0
        mask[p, 104 * 13 + 4 * i + 4] = NEG_BIG  # self slot (dt=0, dh=0, dw=0)

    return {
        "p5n": p5n_aug.astype(ml_dtypes.bfloat16),
        "p7na": np.ascontiguousarray(p7n_aug[:128]).astype(ml_dtypes.bfloat16),
        "p7nb": np.ascontiguousarray(p7n_aug[128:]).astype(ml_dtypes.bfloat16),
        "pq": pq.astype(ml_dtypes.bfloat16),
        "pda": pdA.astype(ml_dtypes.bfloat16),
        "pdb": pdB.astype(ml_dtypes.bfloat16),
        "maskbig": mask,
        "validq": validq,
    }, nq7_sum


# ------------------------------------------------------------------
# matmul run decomposition (python-static): for bank gbank, row (qt, qh),
# yield (dt_idx, s0, s1) contiguous pair-slot runs with the same dt.
def _bank_runs(gbank):
    s_lo, s_hi = 4 * gbank, min(4 * gbank + 4, NPAIRS)
    runs = []
    s = s_lo
    while s < s_hi:
        dt_idx = s // 9
        e = min(s_hi, 9 * (dt_idx + 1))
        runs.append((dt_idx, s, e))
        s = e
    return runs


def build_bass_program():
    import concourse.bass as bass
    import concourse.tile as tile
    from concourse.tile import add_dep_helper
    from concourse import mybir

    fp32 = mybir.dt.float32
    bf16 = mybir.dt.bfloat16

    nc = bass.Bass()
    d_p5n = nc.declare_dram_parameter("p5n", [D5 + 1, T, CH_BAND, WTILD], bf16, isOutput=False)
    d_p7na = nc.declare_dram_parameter("p7na", [128, T, CH_BAND, WTILD], bf16, isOutput=False)
    d_p7nb = nc.declare_dram_parameter("p7nb", [20, T, CH_BAND, WTILD], bf16, isOutput=False)
    d_pq = nc.declare_dram_parameter("pq", [D5 + 1, NTILES, 128], bf16, isOutput=False)
    d_pda = nc.declare_dram_parameter("pda", [128, NTILES, 128], bf16, isOutput=False)
    d_pdb = nc.declare_dram_parameter("pdb", [20, NTILES, 128], bf16, isOutput=False)
    d_mask = nc.declare_dram_parameter("maskbig", [128, XW], fp32, isOutput=False)
    d_vq = nc.declare_dram_parameter("validq", [128, 2], fp32, isOutput=False)
    d_out = nc.declare_dram_parameter("out", [1, 1], fp32, isOutput=True)

    rows = _core_rows(0)  # qt/ch~-geometry identical across cores (band-local)
    prev_mm = [None]

    def _chain(inst):
        # pin PE issue order = program order so PSUM accumulation groups
        # (start/stop pairs) are never interleaved within a bank
        if prev_mm[0] is not None:
            add_dep_helper(inst.ins, prev_mm[0].ins, sync=False, reason="pe order")
        prev_mm[0] = inst

    with tile.TileContext(nc) as tc:
        with (
            tc.tile_pool(name="const", bufs=1) as cpool,
            tc.tile_pool(name="psum", bufs=7, space="PSUM") as pspool,
            tc.tile_pool(name="psfin", bufs=1, space="PSUM") as psfin,
            tc.tile_pool(name="work", bufs=2) as wpool,
            tc.tile_pool(name="small", bufs=2) as spool,
        ):
            # ---- resident loads ----
            p5n = cpool.tile([D5 + 1, T, CH_BAND, WTILD], bf16)
            nc.sync.dma_start(out=p5n[:], in_=d_p5n[:])
            p7na = cpool.tile([128, T, CH_BAND, WTILD], bf16)
            nc.sync.dma_start(out=p7na[:], in_=d_p7na[:])
            # the K-split pair (p7nb weights + rhs) parked at partitions 96..115
            # (matmul requires Fmap and Weight to share the base partition)
            p7nb_h = cpool.tile([116, T, CH_BAND, WTILD], bf16)
            p7nb = p7nb_h[96:116]
            nc.sync.dma_start(out=p7nb[:], in_=d_p7nb[:])
            pq = cpool.tile([D5 + 1, NTILES, 128], bf16)
            nc.sync.dma_start(out=pq[:], in_=d_pq[:])
            pda = cpool.tile([128, NTILES, 128], bf16)
            nc.sync.dma_start(out=pda[:], in_=d_pda[:])
            pdb_h = cpool.tile([116, NTILES, 128], bf16)
            pdb = pdb_h[96:116]
            nc.sync.dma_start(out=pdb[:], in_=d_pdb[:])
            maskbig = cpool.tile([128, XW], fp32)
            nc.sync.dma_start(out=maskbig[:], in_=d_mask[:])

            # one consolidated scratch tile (pool pads each tile to 4KB)
            scr = cpool.tile([128, 32], fp32)
            maxv = scr[:, 0:8]
            m98 = scr[:, 8:16]   # col 8 = m9, cols 9..15 = NEG_ZAP filler
            accs = [scr[:, 16 + g : 17 + g] for g in range(BANKS)]
            total = scr[:, 23:24]
            ones = scr[:, 24:25]
            vq = scr[:, 25:27]
            nc.sync.dma_start(out=vq[:], in_=d_vq[:])
            nc.vector.memset(total[:], 0.0)
            nc.vector.memset(ones[:], 1.0)

            x = cpool.tile([128, XW], fp32)

            for ti in range(NTILES):
                # ---------- search scores into PSUM ----------
                ps_banks = []
                for gb in range(BANKS):
                    npair = min(4 * gb + 4, NPAIRS) - 4 * gb
                    psb = pspool.tile([128, BANKW], fp32, tag="ps")
                    ps_banks.append((psb, npair))
                    for r in range(ROWS_PER_TILE):
                        qt, qh = rows[ti * ROWS_PER_TILE + r]
                        j = (qh % 24) // 4  # band-local row index
                        for dt_idx, s0, s1 in _bank_runs(gb):
                            tprime = min(max(qt + dt_idx - 1, 0), T - 1)
                            lch0 = 4 * j + (s0 - 9 * dt_idx)
                            nrun = s1 - s0
                            rhs = p5n[:, tprime, lch0 : lch0 + nrun, :]
                            out = psb[32 * r : 32 * r + 32,
                                      (s0 - 4 * gb) * WTILD : (s1 - 4 * gb) * WTILD]
                            _chain(nc.tensor.matmul(
                                out, pq[:, ti, 32 * r : 32 * r + 32], rhs,
                                start=True, stop=True, tile_position=(0, 32 * r),
                            ))
                    # ---------- evacuate + mask ----------
                    xs = x[:, 4 * gb * WTILD : (4 * gb + npair) * WTILD]
                    ms = maskbig[:, 4 * gb * WTILD : (4 * gb + npair) * WTILD]
                    nc.vector.tensor_tensor(
                        out=xs, in0=psb[:, 0 : npair * WTILD], in1=ms,
                        op=mybir.AluOpType.add,
                    )

                # ---------- top-9 selection on x ----------
                nc.vector.max(out=maxv[:], in_=x[:])
                nc.vector.match_replace(
                    out=x[:], in_to_replace=maxv[:], in_values=x[:],
                    imm_value=NEG_ZAP,
                )
                nc.vector.memset(m98[:], NEG_ZAP)
                nc.vector.tensor_reduce(
                    out=m98[:, 0:1], in_=x[:], axis=mybir.AxisListType.X,
                    op=mybir.AluOpType.max,
                )
                nc.vector.match_replace(
                    out=x[:], in_to_replace=m98[:], in_values=x[:],
                    imm_value=NEG_ZAP,
                )
                # in place: x becomes the 0/1 selection mask
                nc.vector.tensor_scalar(
                    x[:], x[:], NEG_THR, None, mybir.AluOpType.is_lt
                )

                # ---------- refine scores + masked accumulate ----------
                for gb in range(BANKS):
                    npair = min(4 * gb + 4, NPAIRS) - 4 * gb
                    prb = pspool.tile([128, BANKW], fp32, tag="ps")
                    for r in range(ROWS_PER_TILE):
                        qt, qh = rows[ti * ROWS_PER_TILE + r]
                        j = (qh % 24) // 4
                        for dt_idx, s0, s1 in _bank_runs(gb):
                            tprime = min(max(qt + dt_idx - 1, 0), T - 1)
                            lch0 = 4 * j + (s0 - 9 * dt_idx)
                            nrun = s1 - s0
                            rhsA = p7na[:, tprime, lch0 : lch0 + nrun, :]
                            rhsB = p7nb[:, tprime, lch0 : lch0 + nrun, :]
                            out = prb[32 * r : 32 * r + 32,
                                      (s0 - 4 * gb) * WTILD : (s1 - 4 * gb) * WTILD]
                            lhsA = pda[:, ti, 32 * r : 32 * r + 32]
                            lhsB = pdb[:, ti, 32 * r : 32 * r + 32]
                            _chain(nc.tensor.matmul(
                                out, lhsA, rhsA, start=True,
                                stop=False, tile_position=(0, 32 * r)))
                            _chain(nc.tensor.matmul(
                                out, lhsB, rhsB, start=False,
                                stop=True, tile_position=(96, 32 * r)))
                    sel = x[:, 4 * gb * WTILD : (4 * gb + npair) * WTILD]
                    nc.vector.scalar_tensor_tensor(
                        out=sel,
                        in0=sel,
                        scalar=1.0,
                        in1=prb[:, 0 : npair * WTILD],
                        op0=mybir.AluOpType.bypass,
                        op1=mybir.AluOpType.mult,
                        accum_out=accs[gb][:],
                    )

                # total += sum_g(acc_g) * validq(tile variant)
                accsum = scr[:, 27:28]
                nc.vector.tensor_reduce(
                    out=accsum, in_=scr[:, 16 : 16 + BANKS],
                    axis=mybir.AxisListType.X, op=mybir.AluOpType.add,
                )
                v = vq[:, 1:2] if ti == NTILES - 1 else vq[:, 0:1]
                nc.vector.scalar_tensor_tensor(
                    out=total[:], in0=accsum, scalar=v,
                    in1=total[:], op0=mybir.AluOpType.mult,
                    op1=mybir.AluOpType.add,
                )

            # ---- final partition reduce: out[0,0] = sum_p total[p] ----
            fin = psfin.tile([128, BANKW], fp32, tag="fin")
            _chain(nc.tensor.matmul(fin[0:1, 0:1], ones[:], total[:],
                                    start=True, stop=True))
            osb = spool.tile([1, 1], fp32, tag="osb")
            nc.vector.tensor_copy(out=osb[:], in_=fin[0:1, 0:1])
            nc.sync.dma_start(out=d_out[:], in_=osb[:])

    return nc


_CACHED = {}


def _split_multiwaits(bir_bytes):
    """This walrus build supports one sem-wait per instruction; Tile emits
    several. Split extras onto NoOps inserted immediately before (same
    engine, program order preserved -> semantically identical)."""
    import json

    m = json.loads(bir_bytes)
    ctr = [0]
    for fn in m["functions"]:
        for bb in fn["blocks"]:
            out = []
            for ins in bb["instructions"]:
                si = ins.get("sync_info")
                waits = (si or {}).get("on_wait") or []
                for w in waits[:-1]:
                    ctr[0] += 1
                    out.append(
                        {
                            "debug": ins.get("debug", 0),
                            "engine": ins["engine"],
                            "ins": [],
                            "outs": [],
                            "name": f"I-mwsplit{ctr[0]}",
                            "opcode": "NoOp",
                            "sync_info": {"on_update": [], "on_wait": [w]},
                        }
                    )
                if len(waits) > 1:
                    si["on_wait"] = [waits[-1]]
                out.append(ins)
            bb["instructions"] = out
    return json.dumps(m).encode()


def _install_multiwait_patch():
    if _CACHED.get("mw_patch"):
        return
    from concourse import bass2jax, bass_utils

    orig = bass_utils.compile_bir_kernel

    def wrapper(bir_json, tmpdir, neff_name="file.neff"):
        return orig(_split_multiwaits(bir_json), tmpdir, neff_name=neff_name)

    bass2jax.compile_bir_kernel = wrapper
    _CACHED["mw_patch"] = True


def kernel(noisy, deno, curr_epoch=None):
    noisy = np.asarray(noisy, np.float32)
    deno = np.asarray(deno, np.float32)

    in_maps = []
    nq7_sums = []
    for core in range(NCORES):
        b, g = core % B, core // B
        m, nq7s = build_core_inputs(noisy, deno, b, g)
        in_maps.append(m)
        nq7_sums.append(nq7s)

    if "nc" not in _CACHED:
        _CACHED["nc"] = build_bass_program()
    nc = _CACHED["nc"]

    _install_multiwait_patch()
    from concourse.bass_utils import run_bass_kernel_spmd

    res = run_bass_kernel_spmd(nc, in_maps, list(range(NCORES)))
    parts = [float(res.results[i]["out"].reshape(-1)[0]) for i in range(NCORES)]
    loss = (sum(parts) + sum(nq7_sums)) / (B * T * NH * NW * (K - 1))
    return np.float32(loss)
